# revision 7
# baseline (speedup 1.0000x reference)
"""Trainium2 Bass kernel for the batched constant-velocity Kalman filter.

Key structure exploited:
  * The Kalman covariance recursion is data-independent, so the per-step
    gains and output stats (sx, sy, rho) are batch-wide scalars computed on
    host. rho is exactly 0 (x/y decoupled), and sx == sy.
  * Only the state mean is per-trajectory work: a short scalar-gain
    recursion over 9 observation steps, then a closed-form linear
    extrapolation for the prediction steps.
  * Output is [T_est+len_pred, B, 5] = ~102 MB -> the kernel is dominated
    by the output DMA writes; compute (DVE/ACT elementwise) hides under it.

Sharding: pure data parallel over batch, B=131072 -> 16384 per core x 8.

Per-core layout: batch shard as [128 partitions x 128 lanes], b = p*128 + j.
x/y channels stay interleaved: state tiles are [128, 256] = (j, c) pairs, so
each vector op processes both channels at once. Output steps are grouped
G=4 per SBUF tile [128, 4*640] and written with one contiguous-run DMA per
group (2560 B runs per partition per step), alternating the two HWDGE rings.
The estimation recursion writes its position state directly into the output
tiles (strided, f32 two-operand ops are 1x regardless), so no copy ops.
"""

import numpy as np

DT = 0.1
EPS = 0.01
N_CORES = 8
B_FULL = 131072
B_SHARD = B_FULL // N_CORES  # 16384
T_OBS = 10
P = 128                       # SBUF partitions
J = B_SHARD // P              # 128 lanes per partition
G = 4                         # output steps per DMA group


def _scalar_kalman(sigma_a, sigma_obs, sigma_init, n_est, len_pred):
    """Host-side data-independent 2x2 covariance recursion (float64)."""
    sa2 = float(sigma_a) ** 2
    r = float(sigma_obs) ** 2
    F = np.array([[1.0, DT], [0.0, 1.0]])
    Gm = np.array([DT * DT / 2.0, DT])
    Q = sa2 * np.outer(Gm, Gm)
    Pc = (float(sigma_init) ** 2) * np.eye(2)
    a_l, b_l, sx_l = [], [], []
    for _ in range(n_est):
        Pc = F @ Pc @ F.T + Q
        S = Pc[0, 0] + r
        a = Pc[0, 0] / S
        b = Pc[1, 0] / S
        IKH = np.array([[1.0 - a, 0.0], [-b, 1.0]])
        Pc = IKH @ Pc @ IKH.T + r * np.outer([a, b], [a, b])
        a_l.append(a)
        b_l.append(b)
        sx_l.append(np.sqrt(max(Pc[0, 0], EPS * EPS)))
    for _ in range(len_pred):
        Pc = F @ Pc @ F.T + Q
        sx_l.append(np.sqrt(max(Pc[0, 0], EPS * EPS)))
    return np.array(a_l), np.array(b_l), np.array(sx_l)


_CACHE = {}


def _build(sigma_a, sigma_obs, sigma_init, len_pred):
    import concourse.bacc as bacc
    import concourse.mybir as mybir
    import concourse.tile as tile

    AF = mybir.ActivationFunctionType
    OP = mybir.AluOpType
    F32 = mybir.dt.float32

    n_est = T_OBS - 1
    n_out = n_est + len_pred
    a_g, b_g, sx_g = _scalar_kalman(sigma_a, sigma_obs, sigma_init, n_est, len_pred)
    a_g = a_g.astype(np.float32)
    b_g = b_g.astype(np.float32)
    sx_g = sx_g.astype(np.float32)
    dt = float(np.float32(DT))

    # output-step groups: ramp up quickly, then G per group
    groups = []
    t0 = 0
    tail_small = [2, 2]          # last groups kept small to shrink the drain
    for sz in [2, 2]:
        groups.append((t0, sz))
        t0 += sz
    n_tail = sum(tail_small)
    while t0 < n_out - n_tail:
        sz = min(G, n_out - n_tail - t0)
        groups.append((t0, sz))
        t0 += sz
    for sz in tail_small:
        groups.append((t0, sz))
        t0 += sz

    nc = bacc.Bacc(
        "TRN2",
        target_bir_lowering=False,
        debug=False,
        enable_asserts=False,
        num_devices=N_CORES,
    )
    x = nc.dram_tensor("x", [T_OBS, B_SHARD, 2], F32, kind="ExternalInput")
    y = nc.dram_tensor("y", [n_out, B_SHARD, 5], F32, kind="ExternalOutput")
    x_ap = x.ap()
    y_ap = y.ap()

    with tile.TileContext(nc) as tc:
        with (
            tc.tile_pool(name="zp", bufs=1) as zp,
            tc.tile_pool(name="sp", bufs=1) as sp,
            tc.tile_pool(name="gp", bufs=4) as gp,
        ):
            # --- input loads: z0,z1 first on the sync ring (fast path),
            # the rest via gpsimd SWDGE so output DMAs are not queued
            # behind them on the HWDGE rings.
            # per-step input tiles, loaded as 2-step DMAs spread across all
            # three DMA-capable queues so each est step's data arrives early
            # and independently (desc-gen is ~17ns/descriptor, so one big
            # load would both crawl and gate every step on its completion).
            zt = zp.tile([P, T_OBS * 2 * J], F32, name="zt")
            z3 = zt.rearrange("p (s f) -> p s f", s=T_OBS)
            load_eng = [nc.sync, nc.scalar, nc.gpsimd, nc.sync, nc.scalar]
            for half in range(T_OBS // 2):
                eng = load_eng[half % len(load_eng)]
                eng.dma_start(
                    z3[:, 2 * half : 2 * half + 2, :],
                    x_ap[2 * half : 2 * half + 2].rearrange(
                        "s (p j) c -> p s (j c)", p=P
                    ),
                )

            def zv(s):
                """[128, 256] (j,c)-interleaved view of observation step s."""
                return zt[:, s * 2 * J : (s + 1) * 2 * J]

            dummy = sp.tile([P, 2 * J], F32, name="dummy")
            nc.vector.memset(dummy, 0.0)

            # persistent state tiles ((j,c) interleaved)
            pxy0 = sp.tile([P, 2 * J], F32, name="pxy0")   # pos before step 0
            pxy9 = sp.tile([P, 2 * J], F32, name="pxy9")   # pos after last est
            vxy = sp.tile([P, 2 * J], F32, name="vxy")
            pp = sp.tile([P, 2 * J], F32, name="pp")       # predicted pos
            ixy = sp.tile([P, 2 * J], F32, name="ixy")     # innovation

            # init: pos = z0, vel = (z1 - z0)/dt
            nc.vector.tensor_copy(pxy0, zv(0))
            nc.vector.tensor_sub(ixy, zv(1), zv(0))
            nc.vector.tensor_scalar_mul(vxy, ixy, float(np.float32(1.0 / DT)))

            stt_v = nc.vector.scalar_tensor_tensor
            stt_g = nc.gpsimd.scalar_tensor_tensor

            # group tiles are allocated lazily below; pos_view[t] is the
            # strided [128, 128, 2] AP of step t's pos channels inside its
            # group tile (written by the recursion, read by step t+1).
            pos_view = {}
            n_slot_init = 0

            for gi, (t0, sz) in enumerate(groups):
                gt = gp.tile([P, G * 5 * J], F32, name="gt", tag="gt")
                g4 = gt.rearrange("p (t j c) -> p t j c", t=G, c=5)
                if n_slot_init < 4:
                    # first occupant of each of the 3 slots zeroes the rho
                    # channel over the full G-step range once; later
                    # occupants inherit the zeros (slot memory is stable).
                    nc.vector.memset(g4[:, :, :, 4], 0.0)
                    n_slot_init += 1
                for ti in range(sz):
                    t = t0 + ti
                    opos = g4[:, ti, :, 0:2]
                    # constant channels sx, sy in one fused ACT op
                    nc.scalar.activation(
                        g4[:, ti, :, 2:4], dummy, AF.Copy,
                        bias=float(sx_g[t]), scale=0.0,
                    )
                    if t < n_est:
                        # estimation step t (obs index t+1)
                        prev = pxy0 if t == 0 else pos_view[t - 1]
                        stt_v(pp, vxy, dt, prev, OP.mult, OP.add)
                        nc.vector.tensor_sub(ixy, zv(t + 1), pp)
                        stt_v(opos, ixy, float(a_g[t]), pp, OP.mult, OP.add)
                        stt_v(vxy, ixy, float(b_g[t]), vxy, OP.mult, OP.add)
                        pos_view[t] = opos
                        if t == n_est - 1:
                            # detach final pos state from the group tile so
                            # prediction steps do not pin this slot
                            nc.vector.tensor_copy(pxy9, opos)
                    else:
                        # prediction step: pos = pxy9 + (k*dt)*vxy
                        k = t - n_est + 1
                        kdt = float(np.float32(k) * np.float32(DT))
                        stt_v(opos, vxy, kdt, pxy9, OP.mult, OP.add)
                # one DMA per group, alternating the two HWDGE rings
                eng = nc.sync if gi % 2 == 0 else nc.scalar
                eng.dma_start(
                    y_ap[t0 : t0 + sz].rearrange("t (p j) c -> p t (j c)", p=P),
                    gt.rearrange("p (t f) -> p t f", t=G)[:, :sz, :],
                )

    nc.compile()
    return nc


def kernel(**inputs):
    from concourse import bass_utils

    x_full = np.ascontiguousarray(np.asarray(inputs["inputs"], dtype=np.float32))
    sigma_a = float(np.asarray(inputs["sigma_a"]))
    sigma_obs = float(np.asarray(inputs["sigma_obs"]))
    sigma_init = float(np.asarray(inputs["sigma_init"]))
    len_pred = int(np.asarray(inputs["len_pred"]))
    assert x_full.shape == (T_OBS, B_FULL, 2), x_full.shape

    key = (sigma_a, sigma_obs, sigma_init, len_pred)
    if key not in _CACHE:
        _CACHE[key] = _build(sigma_a, sigma_obs, sigma_init, len_pred)
    nc = _CACHE[key]

    in_maps = [
        {"x": np.ascontiguousarray(x_full[:, c * B_SHARD : (c + 1) * B_SHARD, :])}
        for c in range(N_CORES)
    ]
    res = bass_utils.run_bass_kernel_spmd(nc, in_maps, core_ids=list(range(N_CORES)))
    outs = [r["y"] for r in res.results]
    return np.concatenate(outs, axis=1)


if __name__ == "__main__":
    import ref_np

    inp = ref_np.setup_inputs_np()
    out = kernel(**inp)
    exp = ref_np.reference_np(
        inp["inputs"], inp["sigma_a"], inp["sigma_obs"], inp["sigma_init"],
        int(inp["len_pred"]))
    err = np.abs(out - exp).max()
    print("max abs err vs ref_np:", err, " rel:", err / np.abs(exp).max())


# revision 8
# speedup vs baseline: 1.0550x; 1.0550x over previous
"""Trainium2 Bass kernel for the batched constant-velocity Kalman filter.

Key structure exploited:
  * The Kalman covariance recursion is data-independent, so the per-step
    gains and output stats (sx, sy, rho) are batch-wide scalars computed on
    host. rho is exactly 0 (x/y decoupled), and sx == sy.
  * Only the state mean is per-trajectory work: a short scalar-gain
    recursion over 9 observation steps, then a closed-form linear
    extrapolation for the prediction steps.
  * Output is [T_est+len_pred, B, 5] = ~102 MB -> the kernel is dominated
    by the output DMA writes; compute (DVE/ACT elementwise) hides under it.

Sharding: pure data parallel over batch, B=131072 -> 16384 per core x 8.

Per-core layout: batch shard as [128 partitions x 128 lanes], b = p*128 + j.
x/y channels stay interleaved: state tiles are [128, 256] = (j, c) pairs, so
each vector op processes both channels at once. Output steps are grouped
G=4 per SBUF tile [128, 4*640] and written with one contiguous-run DMA per
group (2560 B runs per partition per step), alternating the two HWDGE rings.
The estimation recursion writes its position state directly into the output
tiles (strided, f32 two-operand ops are 1x regardless), so no copy ops.
"""

import numpy as np

DT = 0.1
EPS = 0.01
N_CORES = 8
B_FULL = 131072
B_SHARD = B_FULL // N_CORES  # 16384
T_OBS = 10
P = 128                       # SBUF partitions
J = B_SHARD // P              # 128 lanes per partition
G = 4                         # output steps per DMA group


def _scalar_kalman(sigma_a, sigma_obs, sigma_init, n_est, len_pred):
    """Host-side data-independent 2x2 covariance recursion (float64)."""
    sa2 = float(sigma_a) ** 2
    r = float(sigma_obs) ** 2
    F = np.array([[1.0, DT], [0.0, 1.0]])
    Gm = np.array([DT * DT / 2.0, DT])
    Q = sa2 * np.outer(Gm, Gm)
    Pc = (float(sigma_init) ** 2) * np.eye(2)
    a_l, b_l, sx_l = [], [], []
    for _ in range(n_est):
        Pc = F @ Pc @ F.T + Q
        S = Pc[0, 0] + r
        a = Pc[0, 0] / S
        b = Pc[1, 0] / S
        IKH = np.array([[1.0 - a, 0.0], [-b, 1.0]])
        Pc = IKH @ Pc @ IKH.T + r * np.outer([a, b], [a, b])
        a_l.append(a)
        b_l.append(b)
        sx_l.append(np.sqrt(max(Pc[0, 0], EPS * EPS)))
    for _ in range(len_pred):
        Pc = F @ Pc @ F.T + Q
        sx_l.append(np.sqrt(max(Pc[0, 0], EPS * EPS)))
    return np.array(a_l), np.array(b_l), np.array(sx_l)


_CACHE = {}


def _build(sigma_a, sigma_obs, sigma_init, len_pred):
    import concourse.bacc as bacc
    import concourse.mybir as mybir
    import concourse.tile as tile

    AF = mybir.ActivationFunctionType
    OP = mybir.AluOpType
    F32 = mybir.dt.float32

    n_est = T_OBS - 1
    n_out = n_est + len_pred
    a_g, b_g, sx_g = _scalar_kalman(sigma_a, sigma_obs, sigma_init, n_est, len_pred)
    a_g = a_g.astype(np.float32)
    b_g = b_g.astype(np.float32)
    sx_g = sx_g.astype(np.float32)
    dt = float(np.float32(DT))

    # output-step groups: ramp up quickly, then G per group
    groups = []
    t0 = 0
    tail_small = [2, 2]          # last groups kept small to shrink the drain
    for sz in [2, 2]:
        groups.append((t0, sz))
        t0 += sz
    n_tail = sum(tail_small)
    while t0 < n_out - n_tail:
        sz = min(G, n_out - n_tail - t0)
        groups.append((t0, sz))
        t0 += sz
    for sz in tail_small:
        groups.append((t0, sz))
        t0 += sz

    nc = bacc.Bacc(
        "TRN2",
        target_bir_lowering=False,
        debug=False,
        enable_asserts=False,
        num_devices=N_CORES,
    )
    x = nc.dram_tensor("x", [P, T_OBS * 2 * J], F32, kind="ExternalInput")
    y = nc.dram_tensor("y", [n_out, B_SHARD, 5], F32, kind="ExternalOutput")
    x_ap = x.ap()
    y_ap = y.ap()

    with tile.TileContext(nc) as tc:
        with (
            tc.tile_pool(name="zp", bufs=1) as zp,
            tc.tile_pool(name="sp", bufs=1) as sp,
            tc.tile_pool(name="gp", bufs=4) as gp,
        ):
            # --- input loads: z0,z1 first on the sync ring (fast path),
            # the rest via gpsimd SWDGE so output DMAs are not queued
            # behind them on the HWDGE rings.
            # input is host-pretransposed to [p, (s j c)] so the whole
            # 1.25 MB shard loads as one DMA with 10 KB contiguous runs
            # (128 descriptors; desc-gen is ~15ns/descriptor, so run length
            # is what sets input bandwidth).
            zt = zp.tile([P, T_OBS * 2 * J], F32, name="zt")
            nc.sync.dma_start(zt, x_ap)

            def zv(s):
                """[128, 256] (j,c)-interleaved view of observation step s."""
                return zt[:, s * 2 * J : (s + 1) * 2 * J]

            dummy = sp.tile([P, 2 * J], F32, name="dummy")
            nc.vector.memset(dummy, 0.0)

            # persistent state tiles ((j,c) interleaved)
            pxy0 = sp.tile([P, 2 * J], F32, name="pxy0")   # pos before step 0
            pxy9 = sp.tile([P, 2 * J], F32, name="pxy9")   # pos after last est
            vxy = sp.tile([P, 2 * J], F32, name="vxy")
            pp = sp.tile([P, 2 * J], F32, name="pp")       # predicted pos
            ixy = sp.tile([P, 2 * J], F32, name="ixy")     # innovation

            # init: pos = z0, vel = (z1 - z0)/dt
            nc.vector.tensor_copy(pxy0, zv(0))
            nc.vector.tensor_sub(ixy, zv(1), zv(0))
            nc.vector.tensor_scalar_mul(vxy, ixy, float(np.float32(1.0 / DT)))

            stt_v = nc.vector.scalar_tensor_tensor
            stt_g = nc.gpsimd.scalar_tensor_tensor

            # group tiles are allocated lazily below; pos_view[t] is the
            # strided [128, 128, 2] AP of step t's pos channels inside its
            # group tile (written by the recursion, read by step t+1).
            pos_view = {}
            n_slot_init = 0

            for gi, (t0, sz) in enumerate(groups):
                gt = gp.tile([P, G * 5 * J], F32, name="gt", tag="gt")
                g4 = gt.rearrange("p (t j c) -> p t j c", t=G, c=5)
                if n_slot_init < 4:
                    # first occupant of each of the 3 slots zeroes the rho
                    # channel over the full G-step range once; later
                    # occupants inherit the zeros (slot memory is stable).
                    nc.vector.memset(g4[:, :, :, 4], 0.0)
                    n_slot_init += 1
                for ti in range(sz):
                    t = t0 + ti
                    opos = g4[:, ti, :, 0:2]
                    # constant channels sx, sy in one fused ACT op
                    nc.scalar.activation(
                        g4[:, ti, :, 2:4], dummy, AF.Copy,
                        bias=float(sx_g[t]), scale=0.0,
                    )
                    if t < n_est:
                        # estimation step t (obs index t+1)
                        prev = pxy0 if t == 0 else pos_view[t - 1]
                        stt_v(pp, vxy, dt, prev, OP.mult, OP.add)
                        nc.vector.tensor_sub(ixy, zv(t + 1), pp)
                        stt_v(opos, ixy, float(a_g[t]), pp, OP.mult, OP.add)
                        stt_v(vxy, ixy, float(b_g[t]), vxy, OP.mult, OP.add)
                        pos_view[t] = opos
                        if t == n_est - 1:
                            # detach final pos state from the group tile so
                            # prediction steps do not pin this slot
                            nc.vector.tensor_copy(pxy9, opos)
                    else:
                        # prediction step: pos = pxy9 + (k*dt)*vxy
                        k = t - n_est + 1
                        kdt = float(np.float32(k) * np.float32(DT))
                        stt_v(opos, vxy, kdt, pxy9, OP.mult, OP.add)
                # one DMA per group, alternating the two HWDGE rings
                eng = nc.sync if gi % 2 == 0 else nc.scalar
                eng.dma_start(
                    y_ap[t0 : t0 + sz].rearrange("t (p j) c -> p t (j c)", p=P),
                    gt.rearrange("p (t f) -> p t f", t=G)[:, :sz, :],
                )

    nc.compile()
    return nc


def kernel(**inputs):
    from concourse import bass_utils

    x_full = np.ascontiguousarray(np.asarray(inputs["inputs"], dtype=np.float32))
    sigma_a = float(np.asarray(inputs["sigma_a"]))
    sigma_obs = float(np.asarray(inputs["sigma_obs"]))
    sigma_init = float(np.asarray(inputs["sigma_init"]))
    len_pred = int(np.asarray(inputs["len_pred"]))
    assert x_full.shape == (T_OBS, B_FULL, 2), x_full.shape

    key = (sigma_a, sigma_obs, sigma_init, len_pred)
    if key not in _CACHE:
        _CACHE[key] = _build(sigma_a, sigma_obs, sigma_init, len_pred)
    nc = _CACHE[key]

    # pre-transpose each core's shard to [p, s, j, c] so the device loads
    # it with long contiguous runs
    x5 = x_full.reshape(T_OBS, N_CORES, P, J, 2)
    in_maps = [
        {"x": np.ascontiguousarray(x5[:, c].transpose(1, 0, 2, 3)).reshape(
            P, T_OBS * 2 * J)}
        for c in range(N_CORES)
    ]
    res = bass_utils.run_bass_kernel_spmd(nc, in_maps, core_ids=list(range(N_CORES)))
    outs = [r["y"] for r in res.results]
    return np.concatenate(outs, axis=1)


if __name__ == "__main__":
    import ref_np

    inp = ref_np.setup_inputs_np()
    out = kernel(**inp)
    exp = ref_np.reference_np(
        inp["inputs"], inp["sigma_a"], inp["sigma_obs"], inp["sigma_init"],
        int(inp["len_pred"]))
    err = np.abs(out - exp).max()
    print("max abs err vs ref_np:", err, " rel:", err / np.abs(exp).max())
